# revision 21
# baseline (speedup 1.0000x reference)
"""Multi-head attention (B=4, S=2048, D=1024, H=16) on 8 trn2 NeuronCores.

Sharding: core c handles batch c//2 and heads (c%2)*8 .. (c%2)*8+8.
Each core computes its partial output through the fc projection; the host
sums the two per-batch partials.

v4: tail reordered (qb order 2,3,1,0 in the last pair) so only tt0-3
wait on the last denominator flush; shuttle DMAs moved to the idle sync
queue; tail output DMAs spread over three queues.  (fp8 q/k projections
were tried and REVERTED: with diffuse attention ctx is a near-cancelled
sum, so the ~2.2% fp8 score noise does NOT wash out relative to ctx —
rel err hit 2.8e-2 vs the 2e-2 gate.)

v3 structure (per core), fp16 operands / fp32 PSUM accumulate:
  - Biasless batched softmax exp: padded keys are zero vectors, so their
    scores are exactly 0 and exp gives 1.0; the ones-column of vhc is a
    VALIDITY column (1 real / 0 padded), so padded keys contribute 0 to
    both numerator and denominator.  One [128,1024] ACTIVATE per
    (pair, q-block, key-chunk) covers both heads of the pair.
  - All DRAM inputs are host-blocked so every load is per-partition
    contiguous on both sides (128 descriptors / DMA, full HBM bw), and
    are issued in dependency-criticality order across the gpsimd / sync /
    scalar DMA queues.
  - Minimal preamble (k/q/v first blocks only); all remaining projection
    and fc work is broken into ~2-matmul micro-ops fed into the attention
    loop as fillers, so the PE array and ACT run concurrently from ~15us.
  - Per-512-q-block denominator shuttle (DRAM roundtrip + partition-
    broadcast read), so normalization never gates more than one q-block.
  - fp16 output; host sums the two per-batch partials in fp32.

PSUM: score pool 2x[128,1024] (4 banks) double-buffered, pv 2 banks,
filler 2 banks.
"""

import numpy as np

import concourse.bass as bass
import concourse.tile as tile
from concourse import mybir
from concourse.bass_utils import run_bass_kernel_spmd

B, S, DM = 4, 2048, 1024
NH, DEPTH = 16, 64
NCORES = 8
HPC = 8                 # heads per core
C = HPC * DEPTH         # 512 output channels per core
SK = 1152               # compacted+padded key count
KC = SK // 128          # 9 key chunks
KB = 3                  # k/v token blocks (384 each)
NQB = S // 512          # 4 q-blocks of 512
DC = DM // 128          # 8 contraction chunks
NPAIR = HPC // 2        # 4 head pairs (= c-tiles of 128)
SCALE = 1.0 / 8.0       # 1/sqrt(depth)

F32 = mybir.dt.float32
FP16 = mybir.dt.float16
EXP = mybir.ActivationFunctionType.Exp


def _split_excess_waits(nc, cap_default=1, cap_evsem=2):
    """walrus in this env rejects >1 sync wait per instruction (2 for event
    semaphores), and FWL-optimized Ldweights rejects any wait; hoist excess
    waits onto preceding same-engine NoOps."""
    n_split = 0
    for f in nc.m.functions:
        for bb in f.blocks:
            insts = list(bb.instructions)
            out = []
            for inst in insts:
                si = inst.sync_info
                if isinstance(inst, mybir.InstLdweights):
                    cap = 0
                elif isinstance(inst, mybir.InstEventSemaphore):
                    cap = cap_evsem
                else:
                    cap = cap_default
                if si is not None and si.on_wait and len(si.on_wait) > cap:
                    waits = list(si.on_wait)
                    if cap == 0:
                        extra, keep = waits, []
                    else:
                        extra, keep = waits[:-cap], waits[-cap:]
                    for i, w in enumerate(extra):
                        nop = mybir.InstNoOp(
                            name=f"{inst.name}_waitsplit_{i}",
                            sync_info=mybir.SyncInfo(on_wait=[w], on_update=[]),
                            bass_nofuse=True,
                            engine=inst.engine,
                        )
                        nc.register_instruction(nop, overwrite=True)
                        out.append(nop)
                    inst.sync_info = mybir.SyncInfo(on_wait=keep, on_update=list(si.on_update))
                    n_split += 1
                out.append(inst)
            if n_split:
                bb.instructions = out
    return n_split


def _emit(tc, t):
    nc = tc.nc
    from contextlib import ExitStack
    ctx = ExitStack()

    persist = ctx.enter_context(tc.tile_pool(name="persist", bufs=1))
    p_a = ctx.enter_context(tc.tile_pool(name="apool", bufs=4))
    p_dinvb = ctx.enter_context(tc.tile_pool(name="dinvb", bufs=3))
    p_small = ctx.enter_context(tc.tile_pool(name="small", bufs=2))
    p_fcr = ctx.enter_context(tc.tile_pool(name="fcr", bufs=8))
    p_out = ctx.enter_context(tc.tile_pool(name="outsb", bufs=6))
    p_s = ctx.enter_context(tc.tile_pool(name="pss", bufs=2, space="PSUM"))
    p_pv = ctx.enter_context(tc.tile_pool(name="pspv", bufs=2, space="PSUM"))
    p_fill = ctx.enter_context(tc.tile_pool(name="psfill", bufs=2, space="PSUM"))

    # persistent buffers (q/k projection operands are fp8 for DoubleRow)
    wq_r = persist.tile([128, DC, C], FP16, tag="wq")
    wk_r = persist.tile([128, DC, C], FP16, tag="wk")
    wv_r = persist.tile([128, DC, C], FP16, tag="wv")
    xq_r = persist.tile([128, NQB, DC, 512], FP16, tag="xq")
    xk_r = persist.tile([128, KB, DC, 384], FP16, tag="xk")
    xv_r = persist.tile([128, KB, DC, 384], FP16, tag="xv")
    qhT = persist.tile([128, NPAIR, S], FP16, tag="qhT")
    khT = persist.tile([128, NPAIR, SK], FP16, tag="khT")
    vhc = persist.tile([128, KC, HPC, DEPTH + 1], FP16, tag="vhc")
    ctxT = persist.tile([128, NPAIR, S], FP16, tag="ctxT")
    maskb = persist.tile([128, KC, 1], F32, tag="maskb")


    # ---- input loads: each DMA queue sustains only ~110 GB/s, so the
    # startup-critical tensors are SPLIT into per-queue chunks (all three
    # queues pull one tensor in parallel) in need-order: kproj needs
    # wk+xk0, then qproj wq+xq0, then vproj wv+xv0; later blocks follow
    # round-robin.  Chunking by the contraction dim also lets the first
    # projection matmuls start as soon as their chunk lands. ----
    engs = [nc.sync, nc.scalar, nc.gpsimd]
    nc.scalar.dma_start(maskb[:], t["maskb"])

    def load3(dst, src, splits=((0, 3), (3, 6), (6, 8))):
        for e, (a, b) in zip(engs, splits):
            e.dma_start(dst[:, a:b], src[:, a:b])

    load3(wk_r, t["wk"])
    load3(xk_r[:, 0], t["xk"][0])
    load3(wq_r, t["wq"])
    load3(xq_r[:, 0], t["xq"][0])
    load3(wv_r, t["wv"])
    load3(xv_r[:, 0], t["xv"][0])
    load3(xk_r[:, 1], t["xk"][1])
    load3(xv_r[:, 1], t["xv"][1])
    load3(xk_r[:, 2], t["xk"][2])
    load3(xv_r[:, 2], t["xv"][2])
    load3(xq_r[:, 1], t["xq"][1])
    load3(xq_r[:, 2], t["xq"][2])
    load3(xq_r[:, 3], t["xq"][3])

    fc_view = t["fcT"].rearrange("(pr p) e -> p pr e", p=128)
    fcrs = []
    for ec in range(2):
        for pair in range(NPAIR):
            fcr = p_fcr.tile([128, 512], FP16, tag="fcr", name=f"fcr_{ec}_{pair}")
            nc.gpsimd.dma_start(fcr[:], fc_view[:, pair, ec * 512:(ec + 1) * 512])
            fcrs.append(fcr)

    # validity column of vhc: 1.0 for real keys, 0.0 for padding
    for h in range(HPC):
        nc.vector.tensor_copy(vhc[:, :, h, DEPTH:DEPTH + 1], maskb[:])

    # ---- micro-op group builders.  Each group owns one PSUM tile for its
    # accumulation; ops emit ~2 matmuls each so fillers interleave finely
    # with the attention stream. ----
    def make_group(pool, tag, width, n_mm, mm_fn, fin_fn, name):
        cell = {}
        ops = []
        n_ops = (n_mm + 1) // 2
        for j in range(n_ops):
            def op(j=j):
                if j == 0:
                    cell["ps"] = pool.tile([128, width], F32, tag=tag, name=name)
                ps = cell["ps"]
                for i in (2 * j, 2 * j + 1):
                    if i < n_mm:
                        mm_fn(ps, i)
                if j == n_ops - 1:
                    fin_fn(ps)
            ops.append(op)
        return ops

    def kproj_ops(pair, kb, pool, tag, width=512):
        def mm(ps, dc):
            nc.tensor.matmul(ps[:, :384],
                             wk_r[:, dc, pair * 128:(pair + 1) * 128],
                             xk_r[:, kb, dc, :],
                             start=(dc == 0), stop=(dc == DC - 1))
        def fin(ps):
            nc.vector.tensor_copy(khT[:, pair, kb * 384:(kb + 1) * 384], ps[:, :384])
        return make_group(pool, tag, width, DC, mm, fin, f"kpj_{pair}_{kb}")

    def qproj_ops(pair, tb, pool, tag, width=512):
        def mm(ps, dc):
            nc.tensor.matmul(ps[:, :512],
                             wq_r[:, dc, pair * 128:(pair + 1) * 128],
                             xq_r[:, tb, dc, :],
                             start=(dc == 0), stop=(dc == DC - 1))
        def fin(ps):
            nc.vector.tensor_copy(qhT[:, pair, tb * 512:(tb + 1) * 512], ps[:, :512])
        return make_group(pool, tag, width, DC, mm, fin, f"qpj_{pair}_{tb}")

    def vproj_ops(kt, pool, tag, width=512):
        kb, off = kt // 3, (kt % 3) * 128
        def mm(ps, dc):
            nc.tensor.matmul(ps[:, :C],
                             xv_r[:, kb, dc, off:off + 128],
                             wv_r[:, dc, :],
                             start=(dc == 0), stop=(dc == DC - 1))
        def fin(ps):
            nc.vector.tensor_copy(
                vhc[:, kt, :, 0:DEPTH],
                ps[:, :C].rearrange("p (h d) -> p h d", h=HPC))
        return make_group(pool, tag, width, DC, mm, fin, f"vpj_{kt}")

    o_view = t["o"].rearrange("(tt p) e -> p tt e", p=128)

    def fc_ops(tt, ec, pool, tag, width=512, copy_eng=None, oeng=None):
        def mm(ps, pair):
            nc.tensor.matmul(ps[:, :512], ctxT[:, pair, tt * 128:(tt + 1) * 128],
                             fcrs[ec * NPAIR + pair][:, :],
                             start=(pair == 0), stop=(pair == NPAIR - 1))
        def fin(ps):
            ob = p_out.tile([128, 512], FP16, tag="outsb", name=f"ob_{tt}_{ec}")
            if copy_eng is nc.scalar:
                nc.scalar.copy(ob[:], ps[:, :512])
            else:
                nc.vector.tensor_copy(ob[:], ps[:, :512])
            eng = oeng or (nc.sync if ec == 0 else nc.gpsimd)
            eng.dma_start(o_view[:, tt, ec * 512:(ec + 1) * 512], ob[:])
        return make_group(pool, tag, width, NPAIR, mm, fin, f"fc_{tt}_{ec}")

    # ---- attention ----
    shuttle_pending = []

    def attention(pair, fillers, qb_order=None, fill_offset=None):
        """fillers: {qb: [micro-ops]} consumed evenly across that qb's steps;
        fill_offset: {qb: first step} to delay consumption."""
        for qb in (qb_order or range(NQB)):
            q0 = qb * 512
            filler = fillers.get(qb, [])
            n_fill = len(filler)
            off = (fill_offset or {}).get(qb, 0)
            span = KC - off + 1    # +1 reserves a share for the post-loop
            fired = 0              # window (PE waits on the last exp there)
            pv = [p_pv.tile([DEPTH + 1, 512], F32, tag="pv",
                            name=f"pv_{pair}_{qb}_{hh}") for hh in range(2)]
            a_prev = None
            for kc in range(KC):
                if kc >= off:
                    want = n_fill * (kc - off + 1) // span
                    while fired < want:
                        filler[fired]()
                        fired += 1
                ps = p_s.tile([128, 1024], F32, tag="s",
                              name=f"s_{pair}_{qb}_{kc}")
                for hh in range(2):
                    lo = 64 * hh
                    nc.tensor.matmul(ps[:, 512 * hh:512 * hh + 512],
                                     khT[lo:lo + 64, pair, kc * 128:(kc + 1) * 128],
                                     qhT[lo:lo + 64, pair, q0:q0 + 512],
                                     start=True, stop=True)
                a_t = p_a.tile([128, 1024], FP16, tag="A",
                               name=f"A_{pair}_{qb}_{kc}")
                nc.scalar.activation(a_t[:], ps[:], EXP, scale=SCALE)
                if kc >= 1:
                    for hh in range(2):
                        nc.tensor.matmul(pv[hh][:], vhc[:, kc - 1, 2 * pair + hh, :],
                                         a_prev[:, 512 * hh:512 * hh + 512],
                                         start=(kc == 1), stop=False)
                a_prev = a_t
            # leftover fillers cover the wait for the last exp
            while fired < n_fill:
                filler[fired]()
                fired += 1
            for hh in range(2):
                nc.tensor.matmul(pv[hh][:], vhc[:, KC - 1, 2 * pair + hh, :],
                                 a_prev[:, 512 * hh:512 * hh + 512],
                                 start=False, stop=True)
            # drain pv: denominator rows onto partition 0 (so the
            # reciprocal runs directly), context out, then an on-chip
            # partition-broadcast + normalize one q-block later.
            dst = p_small.tile([1, 1024], F32, tag="dstg",
                               name=f"dstage_{pair}_{qb}")
            for hh in range(2):
                nc.vector.tensor_copy(dst[0:1, 512 * hh:512 * hh + 512],
                                      pv[hh][DEPTH:DEPTH + 1, :])
                nc.vector.tensor_copy(ctxT[64 * hh:64 * hh + 64, pair, q0:q0 + 512],
                                      pv[hh][0:DEPTH, :])
            # on-chip shuttle: reciprocal on partition 0, then a gpsimd
            # partition-broadcast (SBUF->SBUF, no DRAM roundtrip)
            dinv1 = p_small.tile([1, 1024], F32, tag="dinv1",
                                 name=f"dinv1_{pair}_{qb}")
            nc.vector.reciprocal(dinv1[:], dst[:])

            def stage1(pair=pair, qb=qb, q0=q0, dinv1=dinv1):
                # runs one q-block later: dinv1 has long since landed
                db = p_dinvb.tile([128, 1024], F32, tag="dinvb",
                                  name=f"db_{pair}_{qb}")
                nc.gpsimd.partition_broadcast(db[:], dinv1[:], channels=128)
                for hh in range(2):
                    sl = ctxT[64 * hh:64 * hh + 64, pair, q0:q0 + 512]
                    nc.gpsimd.tensor_mul(
                        sl, sl, db[64 * hh:64 * hh + 64,
                                   512 * hh:512 * hh + 512])
            if shuttle_pending:
                shuttle_pending.pop()()
            shuttle_pending.append(stage1)

    # ---- schedule ----
    # preamble: first k/q blocks only (uses the idle score pool)
    for op in kproj_ops(0, 0, p_s, "s", width=1024):
        op()
    for op in qproj_ops(0, 0, p_s, "s", width=1024):
        op()

    F, FT = p_fill, "fill"
    # qb0 fillers ordered by emission deadline: vhc[kt] must be emitted
    # before PV(kt) (step kt+1), khT block kb before scores kc=3*kb.
    fill0 = (vproj_ops(0, F, FT) + vproj_ops(1, F, FT)
             + kproj_ops(0, 1, F, FT)
             + vproj_ops(2, F, FT) + vproj_ops(3, F, FT)
             + vproj_ops(4, F, FT) + vproj_ops(5, F, FT)
             + kproj_ops(0, 2, F, FT) + vproj_ops(6, F, FT)
             + vproj_ops(7, F, FT) + vproj_ops(8, F, FT)
             + qproj_ops(0, 1, F, FT))
    fill1 = qproj_ops(0, 2, F, FT) + kproj_ops(1, 0, F, FT) + kproj_ops(1, 1, F, FT)
    fill2 = (qproj_ops(0, 3, F, FT) + kproj_ops(1, 2, F, FT)
             + qproj_ops(1, 0, F, FT) + qproj_ops(1, 1, F, FT))
    fill3 = qproj_ops(1, 2, F, FT) + qproj_ops(1, 3, F, FT)
    attention(0, {0: fill0, 1: fill1, 2: fill2, 3: fill3})

    def pair_proj_ops(pair):
        ops = []
        for kb in range(KB):
            ops += kproj_ops(pair, kb, F, FT)
        for tb in range(NQB):
            ops += qproj_ops(pair, tb, F, FT)
        return ops

    for pair in (1, 2):
        nxt = pair_proj_ops(pair + 1)
        nsp = (len(nxt) + 3) // 4
        attention(pair, {qb: nxt[qb * nsp:(qb + 1) * nsp] for qb in range(NQB)})

    # pair 3: qb 2,3 first, then qb1, qb0; each qb's fillers are the fc
    # groups whose tokens were normalized (one q-block late, via the
    # deferred shuttle).  Ending on qb0 means qb1's ctx is normalized at
    # qb0's end, so the tail's first half (tt4-7) is ready immediately and
    # only tt0-3 wait on the final flush.
    def fc_fill(tts):
        return [op for tt in tts for ec in range(2)
                for op in fc_ops(tt, ec, F, FT, oeng=nc.sync)]
    attention(3, {1: fc_fill(range(8, 12)), 0: fc_fill(range(12, 16))},
              qb_order=[2, 3, 1, 0], fill_offset={1: 2, 0: 3})

    # flush the last deferred shuttle stage (normalizes pair3 qb0)
    while shuttle_pending:
        shuttle_pending.pop()()

    # tail: tt4-7 first (ready), then tt0-3 (wait on the flush); alternate
    # pools/copy engines and spread output DMAs over all four queues.
    oengs = [nc.sync, nc.gpsimd, nc.scalar]
    tail = [(tt, ec) for tt in (4, 5, 6, 7, 0, 1, 2, 3) for ec in range(2)]
    for i, (tt, ec) in enumerate(tail):
        pool, tag, w = (p_s, "s", 1024) if i % 2 == 0 else (p_fill, "fill", 512)
        eng = nc.scalar if i % 2 == 0 else nc.vector
        for op in fc_ops(tt, ec, pool, tag, width=w, copy_eng=eng,
                         oeng=oengs[i % 3]):
            op()

    ctx.close()


_NC_CACHE = {}


def _get_nc():
    if "nc" in _NC_CACHE:
        return _NC_CACHE["nc"]
    nc = bass.Bass("TRN2", target_bir_lowering=False, debug=False)
    t = {
        "xq": nc.dram_tensor("xq", (NQB, 128, DC, 512), FP16, kind="ExternalInput").ap(),
        "xk": nc.dram_tensor("xk", (KB, 128, DC, 384), FP16, kind="ExternalInput").ap(),
        "xv": nc.dram_tensor("xv", (KB, 128, DC, 384), FP16, kind="ExternalInput").ap(),
        "wq": nc.dram_tensor("wq", (128, DC, C), FP16, kind="ExternalInput").ap(),
        "wk": nc.dram_tensor("wk", (128, DC, C), FP16, kind="ExternalInput").ap(),
        "wv": nc.dram_tensor("wv", (128, DC, C), FP16, kind="ExternalInput").ap(),
        "fcT": nc.dram_tensor("fcT", (C, DM), FP16, kind="ExternalInput").ap(),
        "maskb": nc.dram_tensor("maskb", (128, KC), F32, kind="ExternalInput").ap(),
        "o": nc.dram_tensor("o", (S, DM), FP16, kind="ExternalOutput").ap(),
    }
    with tile.TileContext(nc) as tc:
        _emit(tc, t)
    _split_excess_waits(nc)
    _NC_CACHE["nc"] = nc
    return nc


def _blk(xT, nblk, blen):
    """[D, S] -> [nblk, 128, DC, blen] with xblk[b, p, dc, s] = xT[dc*128+p, b*blen+s]."""
    D, Sx = xT.shape
    return np.ascontiguousarray(
        xT.reshape(DC, 128, nblk, blen).transpose(2, 1, 0, 3))


def _wblk(wT):
    """[D, C] -> [128, DC, C] with wblk[p, dc, c] = wT[dc*128+p, c]."""
    return np.ascontiguousarray(wT.reshape(DC, 128, C).transpose(1, 0, 2))


def _in_map_for_core(core, v, k, q, mask, wq, wk, wv, fc):
    b = core // 2
    hs = (core % 2) * HPC
    sel = np.nonzero(mask[b] == 0)[0]
    n = len(sel)
    assert n <= SK, f"unmasked key count {n} exceeds static SK={SK}"
    kc_ = np.zeros((SK, DM), np.float16)
    kc_[:n] = k[b][sel]
    vc_ = np.zeros((SK, DM), np.float16)
    vc_[:n] = v[b][sel]
    mb = np.zeros(SK, np.float32)
    mb[:n] = 1.0          # validity: 1 real key, 0 padding
    f16 = np.float16
    return {
        "xq": _blk(q[b].T.astype(f16), NQB, 512),
        "xk": _blk(kc_.T, KB, 384),
        "xv": _blk(vc_.T, KB, 384),
        "wq": _wblk(wq[hs * DEPTH:(hs + HPC) * DEPTH].T.astype(f16)),
        "wk": _wblk(wk[hs * DEPTH:(hs + HPC) * DEPTH].T.astype(f16)),
        "wv": _wblk(wv[hs * DEPTH:(hs + HPC) * DEPTH].T.astype(f16)),
        "fcT": np.ascontiguousarray(fc[:, hs * DEPTH:(hs + HPC) * DEPTH].T.astype(f16)),
        "maskb": np.ascontiguousarray(mb.reshape(KC, 128).T),
    }


def kernel(v, k, q, mask, wq, wk, wv, fc, _run_kwargs=None):
    v = np.asarray(v, np.float32)
    k = np.asarray(k, np.float32)
    q = np.asarray(q, np.float32)
    mask = np.asarray(mask)
    wq = np.asarray(wq, np.float32)
    wk = np.asarray(wk, np.float32)
    wv = np.asarray(wv, np.float32)
    fc = np.asarray(fc, np.float32)

    nc = _get_nc()
    in_maps = [_in_map_for_core(c, v, k, q, mask, wq, wk, wv, fc)
               for c in range(NCORES)]
    res = run_bass_kernel_spmd(nc, in_maps, core_ids=list(range(NCORES)),
                               **(_run_kwargs or {}))
    outs = [r["o"].astype(np.float32) for r in res.results]
    full = np.stack([outs[2 * b] + outs[2 * b + 1] for b in range(B)])
    if _run_kwargs:
        kernel.last_results = res
    return full



# revision 33
# speedup vs baseline: 1.0363x; 1.0363x over previous
"""Multi-head attention (B=4, S=2048, D=1024, H=16) on 8 trn2 NeuronCores.

Sharding: core c handles batch c//2 and heads (c%2)*8 .. (c%2)*8+8.
Each core computes its partial output through the fc projection; the host
sums the two per-batch partials.

v4: tail reordered (qb order 2,3,1,0 in the last pair) so only tt0-3
wait on the last denominator flush; shuttle DMAs moved to the idle sync
queue; tail output DMAs spread over three queues.  (fp8 q/k projections
were tried and REVERTED: with diffuse attention ctx is a near-cancelled
sum, so the ~2.2% fp8 score noise does NOT wash out relative to ctx —
rel err hit 2.8e-2 vs the 2e-2 gate.)

v3 structure (per core), fp16 operands / fp32 PSUM accumulate:
  - Biasless batched softmax exp: padded keys are zero vectors, so their
    scores are exactly 0 and exp gives 1.0; the ones-column of vhc is a
    VALIDITY column (1 real / 0 padded), so padded keys contribute 0 to
    both numerator and denominator.  One [128,1024] ACTIVATE per
    (pair, q-block, key-chunk) covers both heads of the pair.
  - All DRAM inputs are host-blocked so every load is per-partition
    contiguous on both sides (128 descriptors / DMA, full HBM bw), and
    are issued in dependency-criticality order across the gpsimd / sync /
    scalar DMA queues.
  - Minimal preamble (k/q/v first blocks only); all remaining projection
    and fc work is broken into ~2-matmul micro-ops fed into the attention
    loop as fillers, so the PE array and ACT run concurrently from ~15us.
  - Per-512-q-block denominator shuttle (DRAM roundtrip + partition-
    broadcast read), so normalization never gates more than one q-block.
  - fp16 output; host sums the two per-batch partials in fp32.

PSUM: score pool 2x[128,1024] (4 banks) double-buffered, pv 2 banks,
filler 2 banks.
"""

import numpy as np

import concourse.bass as bass
import concourse.tile as tile
from concourse import mybir
from concourse.bass_utils import run_bass_kernel_spmd

B, S, DM = 4, 2048, 1024
NH, DEPTH = 16, 64
NCORES = 8
HPC = 8                 # heads per core
C = HPC * DEPTH         # 512 output channels per core
SK = 1152               # compacted+padded key count
KC = SK // 128          # 9 key chunks
KB = 3                  # k/v token blocks (384 each)
NQB = S // 512          # 4 q-blocks of 512
DC = DM // 128          # 8 contraction chunks
NPAIR = HPC // 2        # 4 head pairs (= c-tiles of 128)
SCALE = 1.0 / 8.0       # 1/sqrt(depth)

F32 = mybir.dt.float32
FP16 = mybir.dt.float16
EXP = mybir.ActivationFunctionType.Exp


def _split_excess_waits(nc, cap_default=1, cap_evsem=2):
    """walrus in this env rejects >1 sync wait per instruction (2 for event
    semaphores), and FWL-optimized Ldweights rejects any wait; hoist excess
    waits onto preceding same-engine NoOps."""
    n_split = 0
    for f in nc.m.functions:
        for bb in f.blocks:
            insts = list(bb.instructions)
            out = []
            for inst in insts:
                si = inst.sync_info
                if isinstance(inst, mybir.InstLdweights):
                    cap = 0
                elif isinstance(inst, mybir.InstEventSemaphore):
                    cap = cap_evsem
                else:
                    cap = cap_default
                if si is not None and si.on_wait and len(si.on_wait) > cap:
                    waits = list(si.on_wait)
                    if cap == 0:
                        extra, keep = waits, []
                    else:
                        extra, keep = waits[:-cap], waits[-cap:]
                    for i, w in enumerate(extra):
                        nop = mybir.InstNoOp(
                            name=f"{inst.name}_waitsplit_{i}",
                            sync_info=mybir.SyncInfo(on_wait=[w], on_update=[]),
                            bass_nofuse=True,
                            engine=inst.engine,
                        )
                        nc.register_instruction(nop, overwrite=True)
                        out.append(nop)
                    inst.sync_info = mybir.SyncInfo(on_wait=keep, on_update=list(si.on_update))
                    n_split += 1
                out.append(inst)
            if n_split:
                bb.instructions = out
    return n_split


def _emit(tc, t):
    nc = tc.nc
    from contextlib import ExitStack
    ctx = ExitStack()

    persist = ctx.enter_context(tc.tile_pool(name="persist", bufs=1))
    p_a = ctx.enter_context(tc.tile_pool(name="apool", bufs=4))
    p_dinvb = ctx.enter_context(tc.tile_pool(name="dinvb", bufs=3))
    p_small = ctx.enter_context(tc.tile_pool(name="small", bufs=4))
    p_fcr = ctx.enter_context(tc.tile_pool(name="fcr", bufs=8))
    p_out = ctx.enter_context(tc.tile_pool(name="outsb", bufs=6))
    p_s = ctx.enter_context(tc.tile_pool(name="pss", bufs=2, space="PSUM"))
    p_pv = ctx.enter_context(tc.tile_pool(name="pspv", bufs=2, space="PSUM"))
    p_fill = ctx.enter_context(tc.tile_pool(name="psfill", bufs=2, space="PSUM"))

    # persistent buffers (q/k projection operands are fp8 for DoubleRow)
    wq_r = persist.tile([128, DC, C], FP16, tag="wq")
    wk_r = persist.tile([128, DC, C], FP16, tag="wk")
    wv_r = persist.tile([128, DC, C], FP16, tag="wv")
    xq_r = persist.tile([128, NQB, DC, 512], FP16, tag="xq")
    xk_r = persist.tile([128, KB, DC, 384], FP16, tag="xk")
    xv_r = persist.tile([128, KB, DC, 384], FP16, tag="xv")
    qhT = persist.tile([128, NPAIR, S], FP16, tag="qhT")
    khT = persist.tile([128, NPAIR, SK], FP16, tag="khT")
    vhc = persist.tile([128, KC, HPC, DEPTH + 1], FP16, tag="vhc")
    ctxT = persist.tile([128, NPAIR, S], FP16, tag="ctxT")
    maskb = persist.tile([128, KC, 1], F32, tag="maskb")


    # ---- input loads.  All DMAs share one ~16-engine hardware pool that
    # services queued transfers round-robin, so anything enqueued early
    # steals bandwidth from the startup-critical tensors.  Only the
    # preamble's tensors (kproj: wk+xk0, qproj: wq+xq0, vproj: wv+xv0,
    # then the kb1 blocks) are issued up front; everything else is issued
    # later from the scalar engine's exp stream (pre_act hooks below) or
    # the gpsimd stream, which are naturally paced by the schedule. ----
    nc.scalar.dma_start(maskb[:], t["maskb"])
    nc.sync.dma_start(wk_r[:], t["wk"])
    nc.scalar.dma_start(xk_r[:, 0], t["xk"][0])
    nc.gpsimd.dma_start(wq_r[:], t["wq"])
    nc.sync.dma_start(xq_r[:, 0], t["xq"][0])
    nc.scalar.dma_start(wv_r[:], t["wv"])
    nc.gpsimd.dma_start(xv_r[:, 0], t["xv"][0])
    nc.scalar.dma_start(xk_r[:, 1], t["xk"][1])
    nc.scalar.dma_start(xv_r[:, 1], t["xv"][1])

    # deferred loads fired from inside the attention schedule
    def dload(dst, src):
        return lambda: nc.scalar.dma_start(dst, src)

    deferred = {
        (0, 1): dload(xq_r[:, 1], t["xq"][1]),   # qproj(0,1) fires ~qb0 end
        (0, 3): dload(xk_r[:, 2], t["xk"][2]),   # kproj(0,2) fires ~kc6
        (0, 4): dload(xv_r[:, 2], t["xv"][2]),   # vproj kb2 fires ~kc7
        (0, 6): dload(xq_r[:, 2], t["xq"][2]),   # qproj(0,2) fires at qb1 kc0
        (1, 0): dload(xq_r[:, 3], t["xq"][3]),   # qproj(0,3) fires at qb2 kc0
    }

    fc_view = t["fcT"].rearrange("(pr p) e -> p pr e", p=128)
    fcrs = [p_fcr.tile([128, 512], FP16, tag="fcr", name=f"fcr_{ec}_{pair}")
            for ec in range(2) for pair in range(NPAIR)]

    def load_fcrs():
        for i, fcr in enumerate(fcrs):
            ec, pair = divmod(i, NPAIR)
            nc.gpsimd.dma_start(fcr[:], fc_view[:, pair, ec * 512:(ec + 1) * 512])

    # validity column of vhc: 1.0 for real keys, 0.0 for padding
    for h in range(HPC):
        nc.vector.tensor_copy(vhc[:, :, h, DEPTH:DEPTH + 1], maskb[:])

    # internal DRAM bounce for the denominator shuttle: logical block
    # r = pair*4+qb; within it, rows 0-3 = head0's 512 q, rows 4-7 = head1's
    d_dram = nc.dram_tensor("d_dram", (NPAIR * NQB * 8, 128), F32,
                            kind="Internal").ap()
    d_wview = d_dram.rearrange("(r j) f -> r (j f)", j=8)
    dinv_dram = nc.dram_tensor("dinv_dram", (NPAIR * NQB * 8, 128), F32,
                               kind="Internal").ap()
    dinv_flat = dinv_dram.rearrange("a b -> (a b)")

    # ---- micro-op group builders.  Each group owns one PSUM tile for its
    # accumulation; ops emit ~2 matmuls each so fillers interleave finely
    # with the attention stream. ----
    def make_group(pool, tag, width, n_mm, mm_fn, fin_fn, name):
        cell = {}
        ops = []
        n_ops = (n_mm + 1) // 2
        for j in range(n_ops):
            def op(j=j):
                if j == 0:
                    cell["ps"] = pool.tile([128, width], F32, tag=tag, name=name)
                ps = cell["ps"]
                for i in (2 * j, 2 * j + 1):
                    if i < n_mm:
                        mm_fn(ps, i)
                if j == n_ops - 1:
                    fin_fn(ps)
            ops.append(op)
        return ops

    def kproj_ops(pair, kb, pool, tag, width=512):
        def mm(ps, dc):
            nc.tensor.matmul(ps[:, :384],
                             wk_r[:, dc, pair * 128:(pair + 1) * 128],
                             xk_r[:, kb, dc, :],
                             start=(dc == 0), stop=(dc == DC - 1))
        def fin(ps):
            nc.vector.tensor_copy(khT[:, pair, kb * 384:(kb + 1) * 384], ps[:, :384])
        return make_group(pool, tag, width, DC, mm, fin, f"kpj_{pair}_{kb}")

    def qproj_ops(pair, tb, pool, tag, width=512):
        def mm(ps, dc):
            nc.tensor.matmul(ps[:, :512],
                             wq_r[:, dc, pair * 128:(pair + 1) * 128],
                             xq_r[:, tb, dc, :],
                             start=(dc == 0), stop=(dc == DC - 1))
        def fin(ps):
            nc.vector.tensor_copy(qhT[:, pair, tb * 512:(tb + 1) * 512], ps[:, :512])
        return make_group(pool, tag, width, DC, mm, fin, f"qpj_{pair}_{tb}")

    def vproj_ops(kt, pool, tag, width=512):
        kb, off = kt // 3, (kt % 3) * 128
        def mm(ps, dc):
            nc.tensor.matmul(ps[:, :C],
                             xv_r[:, kb, dc, off:off + 128],
                             wv_r[:, dc, :],
                             start=(dc == 0), stop=(dc == DC - 1))
        def fin(ps):
            nc.vector.tensor_copy(
                vhc[:, kt, :, 0:DEPTH],
                ps[:, :C].rearrange("p (h d) -> p h d", h=HPC))
        return make_group(pool, tag, width, DC, mm, fin, f"vpj_{kt}")

    o_view = t["o"].rearrange("(tt p) e -> p tt e", p=128)

    def fc_ops(tt, ec, pool, tag, width=512, copy_eng=None, oeng=None):
        def mm(ps, pair):
            nc.tensor.matmul(ps[:, :512], ctxT[:, pair, tt * 128:(tt + 1) * 128],
                             fcrs[ec * NPAIR + pair][:, :],
                             start=(pair == 0), stop=(pair == NPAIR - 1))
        def fin(ps):
            ob = p_out.tile([128, 512], FP16, tag="outsb", name=f"ob_{tt}_{ec}")
            if copy_eng is nc.scalar:
                nc.scalar.copy(ob[:], ps[:, :512])
            else:
                nc.vector.tensor_copy(ob[:], ps[:, :512])
            eng = oeng or (nc.sync if ec == 0 else nc.gpsimd)
            eng.dma_start(o_view[:, tt, ec * 512:(ec + 1) * 512], ob[:])
        return make_group(pool, tag, width, NPAIR, mm, fin, f"fc_{tt}_{ec}")

    # ---- attention ----
    shuttle_pending = []

    def attention(pair, fillers, qb_order=None, fill_offset=None, pre_act=None):
        """fillers: {qb: [micro-ops]} consumed evenly across that qb's steps;
        fill_offset: {qb: first step} to delay consumption; pre_act:
        {(qb, kc): hook} runs just before that step's exp (used to issue
        deferred input DMAs from the scalar engine's paced stream)."""
        for qb in (qb_order or range(NQB)):
            q0 = qb * 512
            filler = fillers.get(qb, [])
            n_fill = len(filler)
            off = (fill_offset or {}).get(qb, 0)
            span = KC - off + 1    # +1 reserves a share for the post-loop
            fired = 0              # window (PE waits on the last exp there)
            pv = [p_pv.tile([DEPTH + 1, 512], F32, tag="pv",
                            name=f"pv_{pair}_{qb}_{hh}") for hh in range(2)]
            a_prev = None
            for kc in range(KC):
                if kc >= off:
                    want = n_fill * (kc - off + 1) // span
                    while fired < want:
                        filler[fired]()
                        fired += 1
                ps = p_s.tile([128, 1024], F32, tag="s",
                              name=f"s_{pair}_{qb}_{kc}")
                for hh in range(2):
                    lo = 64 * hh
                    nc.tensor.matmul(ps[:, 512 * hh:512 * hh + 512],
                                     khT[lo:lo + 64, pair, kc * 128:(kc + 1) * 128],
                                     qhT[lo:lo + 64, pair, q0:q0 + 512],
                                     start=True, stop=True)
                hook = (pre_act or {}).get((qb, kc))
                if hook is not None:
                    hook()
                a_t = p_a.tile([128, 1024], FP16, tag="A",
                               name=f"A_{pair}_{qb}_{kc}")
                nc.scalar.activation(a_t[:], ps[:], EXP, scale=SCALE)
                if kc >= 1:
                    for hh in range(2):
                        nc.tensor.matmul(pv[hh][:], vhc[:, kc - 1, 2 * pair + hh, :],
                                         a_prev[:, 512 * hh:512 * hh + 512],
                                         start=(kc == 1), stop=False)
                a_prev = a_t
            # leftover fillers cover the wait for the last exp
            while fired < n_fill:
                filler[fired]()
                fired += 1
            for hh in range(2):
                nc.tensor.matmul(pv[hh][:], vhc[:, KC - 1, 2 * pair + hh, :],
                                 a_prev[:, 512 * hh:512 * hh + 512],
                                 start=False, stop=True)
            # drain pv: denominator rows onto partition 0 (so the
            # reciprocal runs directly), context out, then an on-chip
            # partition-broadcast + normalize one q-block later.
            dst = p_small.tile([1, 1024], F32, tag="dstg",
                               name=f"dstage_{pair}_{qb}")
            for hh in range(2):
                nc.vector.tensor_copy(dst[0:1, 512 * hh:512 * hh + 512],
                                      pv[hh][DEPTH:DEPTH + 1, :])
                nc.vector.tensor_copy(ctxT[64 * hh:64 * hh + 64, pair, q0:q0 + 512],
                                      pv[hh][0:DEPTH, :])
            # shuttle stage 0: bounce D to DRAM and regroup onto 8 partitions
            # (issued on the lightly-loaded sync queue)
            r0 = pair * NQB + qb
            nc.sync.dma_start(d_wview[r0:r0 + 1, :], dst[:])
            d8 = p_small.tile([8, 128], F32, tag="d8", name=f"d8_{pair}_{qb}")
            nc.sync.dma_start(d8[:], d_dram[8 * r0:8 * r0 + 8, :])

            def stage1(pair=pair, qb=qb, q0=q0, r0=r0, d8=d8):
                # runs one q-block later: every input has long since landed
                dinv = p_small.tile([8, 128], F32, tag="dinv",
                                    name=f"dinv_{pair}_{qb}")
                nc.vector.reciprocal(dinv[:], d8[:])
                nc.gpsimd.dma_start(dinv_dram[8 * r0:8 * r0 + 8, :], dinv[:])
                db = p_dinvb.tile([128, 512], F32, tag="dinvb",
                                  name=f"db_{pair}_{qb}")
                for hh in range(2):
                    off_d = r0 * 1024 + hh * 512
                    nc.gpsimd.dma_start(
                        db[64 * hh:64 * hh + 64, :],
                        dinv_flat[off_d:off_d + 512].partition_broadcast(64))
                for hh in range(2):
                    sl = ctxT[64 * hh:64 * hh + 64, pair, q0:q0 + 512]
                    nc.gpsimd.tensor_mul(sl, sl, db[64 * hh:64 * hh + 64, :])
            if shuttle_pending:
                shuttle_pending.pop()()
            shuttle_pending.append(stage1)

    # ---- schedule ----
    # preamble: first k/q blocks only (uses the idle score pool)
    for op in kproj_ops(0, 0, p_s, "s", width=1024):
        op()
    for op in qproj_ops(0, 0, p_s, "s", width=1024):
        op()

    F, FT = p_fill, "fill"
    # qb0 fillers ordered by emission deadline: vhc[kt] must be emitted
    # before PV(kt) (step kt+1), khT block kb before scores kc=3*kb.
    fill0 = (vproj_ops(0, F, FT) + vproj_ops(1, F, FT)
             + kproj_ops(0, 1, F, FT)
             + vproj_ops(2, F, FT) + vproj_ops(3, F, FT)
             + vproj_ops(4, F, FT) + vproj_ops(5, F, FT)
             + kproj_ops(0, 2, F, FT) + vproj_ops(6, F, FT)
             + vproj_ops(7, F, FT) + vproj_ops(8, F, FT)
             + qproj_ops(0, 1, F, FT))
    fill1 = qproj_ops(0, 2, F, FT) + kproj_ops(1, 0, F, FT) + kproj_ops(1, 1, F, FT)
    fill2 = (qproj_ops(0, 3, F, FT) + kproj_ops(1, 2, F, FT)
             + qproj_ops(1, 0, F, FT) + qproj_ops(1, 1, F, FT))
    fill3 = qproj_ops(1, 2, F, FT) + qproj_ops(1, 3, F, FT)
    attention(0, {0: fill0, 1: fill1, 2: fill2, 3: fill3}, pre_act=deferred)

    def pair_proj_ops(pair):
        ops = []
        for kb in range(KB):
            ops += kproj_ops(pair, kb, F, FT)
        for tb in range(NQB):
            ops += qproj_ops(pair, tb, F, FT)
        return ops

    load_fcrs()    # gpsimd reaches this after pair0's normalizes (~60us)

    for pair in (1, 2):
        nxt = pair_proj_ops(pair + 1)
        nsp = (len(nxt) + 3) // 4
        attention(pair, {qb: nxt[qb * nsp:(qb + 1) * nsp] for qb in range(NQB)})

    # pair 3: qb 2,3 first, then qb1, qb0; each qb's fillers are the fc
    # groups whose tokens were normalized (one q-block late, via the
    # deferred shuttle).  Ending on qb0 means qb1's ctx is normalized at
    # qb0's end, so the tail's first half (tt4-7) is ready immediately and
    # only tt0-3 wait on the final flush.
    def fc_fill(tts):
        return [op for tt in tts for ec in range(2)
                for op in fc_ops(tt, ec, F, FT, oeng=nc.sync)]
    attention(3, {1: fc_fill(range(8, 12)), 0: fc_fill(range(12, 16))},
              qb_order=[2, 3, 1, 0], fill_offset={1: 2, 0: 3})

    # flush the last deferred shuttle stage (normalizes pair3 qb0)
    while shuttle_pending:
        shuttle_pending.pop()()

    # tail: tt4-7 first (ready), then tt0-3 (wait on the flush); alternate
    # pools/copy engines and spread output DMAs over all four queues.
    oengs = [nc.sync, nc.gpsimd, nc.scalar]
    tail = [(tt, ec) for tt in (4, 5, 6, 7, 0, 1, 2, 3) for ec in range(2)]
    for i, (tt, ec) in enumerate(tail):
        pool, tag, w = (p_s, "s", 1024) if i % 2 == 0 else (p_fill, "fill", 512)
        eng = nc.scalar if i % 2 == 0 else nc.vector
        for op in fc_ops(tt, ec, pool, tag, width=w, copy_eng=eng,
                         oeng=oengs[i % 3]):
            op()

    ctx.close()


_NC_CACHE = {}


def _get_nc():
    if "nc" in _NC_CACHE:
        return _NC_CACHE["nc"]
    nc = bass.Bass("TRN2", target_bir_lowering=False, debug=False)
    t = {
        "xq": nc.dram_tensor("xq", (NQB, 128, DC, 512), FP16, kind="ExternalInput").ap(),
        "xk": nc.dram_tensor("xk", (KB, 128, DC, 384), FP16, kind="ExternalInput").ap(),
        "xv": nc.dram_tensor("xv", (KB, 128, DC, 384), FP16, kind="ExternalInput").ap(),
        "wq": nc.dram_tensor("wq", (128, DC, C), FP16, kind="ExternalInput").ap(),
        "wk": nc.dram_tensor("wk", (128, DC, C), FP16, kind="ExternalInput").ap(),
        "wv": nc.dram_tensor("wv", (128, DC, C), FP16, kind="ExternalInput").ap(),
        "fcT": nc.dram_tensor("fcT", (C, DM), FP16, kind="ExternalInput").ap(),
        "maskb": nc.dram_tensor("maskb", (128, KC), F32, kind="ExternalInput").ap(),
        "o": nc.dram_tensor("o", (S, DM), FP16, kind="ExternalOutput").ap(),
    }
    with tile.TileContext(nc) as tc:
        _emit(tc, t)
    _split_excess_waits(nc)
    _NC_CACHE["nc"] = nc
    return nc


def _blk(xT, nblk, blen):
    """[D, S] -> [nblk, 128, DC, blen] with xblk[b, p, dc, s] = xT[dc*128+p, b*blen+s]."""
    D, Sx = xT.shape
    return np.ascontiguousarray(
        xT.reshape(DC, 128, nblk, blen).transpose(2, 1, 0, 3))


def _wblk(wT):
    """[D, C] -> [128, DC, C] with wblk[p, dc, c] = wT[dc*128+p, c]."""
    return np.ascontiguousarray(wT.reshape(DC, 128, C).transpose(1, 0, 2))


def _in_map_for_core(core, v, k, q, mask, wq, wk, wv, fc):
    b = core // 2
    hs = (core % 2) * HPC
    sel = np.nonzero(mask[b] == 0)[0]
    n = len(sel)
    assert n <= SK, f"unmasked key count {n} exceeds static SK={SK}"
    kc_ = np.zeros((SK, DM), np.float16)
    kc_[:n] = k[b][sel]
    vc_ = np.zeros((SK, DM), np.float16)
    vc_[:n] = v[b][sel]
    mb = np.zeros(SK, np.float32)
    mb[:n] = 1.0          # validity: 1 real key, 0 padding
    f16 = np.float16
    return {
        "xq": _blk(q[b].T.astype(f16), NQB, 512),
        "xk": _blk(kc_.T, KB, 384),
        "xv": _blk(vc_.T, KB, 384),
        "wq": _wblk(wq[hs * DEPTH:(hs + HPC) * DEPTH].T.astype(f16)),
        "wk": _wblk(wk[hs * DEPTH:(hs + HPC) * DEPTH].T.astype(f16)),
        "wv": _wblk(wv[hs * DEPTH:(hs + HPC) * DEPTH].T.astype(f16)),
        "fcT": np.ascontiguousarray(fc[:, hs * DEPTH:(hs + HPC) * DEPTH].T.astype(f16)),
        "maskb": np.ascontiguousarray(mb.reshape(KC, 128).T),
    }


def kernel(v, k, q, mask, wq, wk, wv, fc, _run_kwargs=None):
    v = np.asarray(v, np.float32)
    k = np.asarray(k, np.float32)
    q = np.asarray(q, np.float32)
    mask = np.asarray(mask)
    wq = np.asarray(wq, np.float32)
    wk = np.asarray(wk, np.float32)
    wv = np.asarray(wv, np.float32)
    fc = np.asarray(fc, np.float32)

    nc = _get_nc()
    in_maps = [_in_map_for_core(c, v, k, q, mask, wq, wk, wv, fc)
               for c in range(NCORES)]
    res = run_bass_kernel_spmd(nc, in_maps, core_ids=list(range(NCORES)),
                               **(_run_kwargs or {}))
    outs = [r["o"].astype(np.float32) for r in res.results]
    full = np.stack([outs[2 * b] + outs[2 * b + 1] for b in range(B)])
    if _run_kwargs:
        kernel.last_results = res
    return full

